# revision 18
# baseline (speedup 1.0000x reference)
"""Trainium2 Bass kernel for nn_GCK3x3Layer: 3x3 VALID conv, 256->256 ch, 258x258.

result = kernelsL @ im2col_3x3(input); input (1,256,258,258) f32,
kernelsL (256, 2304) f32 -> output (1, 256, 256, 256) f32.

Strategy: spatial-parallel across 8 NeuronCores. Each core gets a 34-row
input slab (32 output rows + 2 halo rows) and the full weight matrix, and
computes all 256 output channels for its strip via implicit-GEMM:
for each of 9 filter taps and 2 input-channel blocks, a [128,128]x[128,512]
matmul accumulating into PSUM (K = 2304 contraction in 18 chunks of 128,
N = 512 = two output rows of 256 pixels).

Schedule: weight-stationary with LDWEIGHTS dedup. Matmuls are issued
ki-outer / row-pair-inner so 8 consecutive matmuls (into the 8 PSUM
banks) share one weight tile, and a post-compile pass deletes the
duplicate InstLdweights that tile_legalize pairs with every matmul
(576 -> 131). Measured 166.5us/pass/core vs 177.3us for the ki-inner
order with per-matmul weight loads.

Measured perf envelope (this axon-tunneled trn2): the tensor engine
sustains ~1.75 streamed columns/ns (not the nominal 2.4GHz): probes
showed total time tracks total matmul free-dim columns (294,912/core)
regardless of N per matmul (576xN512 == 1152xN256), rhs AP contiguity,
or drain activity - so ~166us is the column-rate floor for bf16.
fp8 DoubleRow would halve columns but fails the 2e-2 gate (measured
e4m3 relmax 0.039; w-split hi+lo 0.026 at 2x cost). Longer sustained
bursts throttle further (repeat=2 measured +12%/pass).
"""

import os
import sys
from contextlib import ExitStack

import numpy as np

for _p in (
    "/root/.axon_site",
    "/root/.axon_site/_ro/trn_rl_repo",
    "/root/.axon_site/_ro/pypackages",
    "/opt/trn_rl_repo",
):
    if os.path.isdir(_p) and _p not in sys.path:
        sys.path.append(_p)

import ml_dtypes  # noqa: E402

import concourse.bass as bass  # noqa: E402
import concourse.tile as tile  # noqa: E402
from concourse import bacc, mybir  # noqa: E402
from concourse.bass_utils import run_bass_kernel_spmd  # noqa: E402

IN_C = 256
OUT_C = 256
H = 258
W = 258
H_OUT = H - 2  # 256
W_OUT = W - 2  # 256
NCORES = 8
ROWS_PER_CORE = H_OUT // NCORES  # 32
IN_ROWS = ROWS_PER_CORE + 2  # 34
P = 128
ICB = IN_C // P  # 2 input-channel blocks
OCB = OUT_C // P  # 2 output-channel blocks
KB = ICB * 9  # 18 contraction blocks of 128
PAIRS = ROWS_PER_CORE // 2  # 16 output-row pairs (N=512 per matmul)

F32 = mybir.dt.float32


def build(
    mm_dtype=mybir.dt.bfloat16,
    repeat=1,
    x_chunk_rows=6,
    loop_repeat=1,
    out_dt=mybir.dt.bfloat16,
    split_queues=True,
    same_weights=False,  # TIMING PROBE ONLY: reuse one weight tile in all
    # matmuls (wrong numerics) to see if repeated identical LDWEIGHTS get
    # elided / hidden. Never used by kernel().
    rows_per_mm=2,  # output rows per matmul: 2 -> N=512 (one PSUM bank),
    # 4 -> N=1024 (PSUM tile spans two banks, halves matmul count).
    # NOTE: 4 is rejected by the ISA (s3d3_mm_num_elements) - matmul
    # output must fit one PSUM bank. Keep 2.
    skip_out=False,  # TIMING PROBE ONLY: drop PSUM->SBUF copies and
    # output stores (wrong output) to bracket the drain-path cost.
    interleave=False,  # interleave the two ocb accumulation groups of each
    # row-pair (two PSUM banks in flight), halving group boundaries so the
    # PE issue stream has fewer chances to micro-idle (HAM oscillation).
    weight_reuse=8,  # None = original order (ki inner per PSUM group).
    # int R: weight-stationary order - for each contraction block ki, issue
    # R matmuls (R row-pair PSUM banks in flight) sharing one weight tile,
    # so consecutive LDWEIGHTS are identical and dedup_ldw can drop them.
    # R=8 claims all 8 PSUM banks; the bank-reuse wait at each of the 4
    # block boundaries trails the last matmul writing that bank by 7
    # matmuls, so the DVE drains never stall the PE. Measured (with
    # dedup_ldw) 166.5us/pass vs 177.3us for the original order; R=4
    # measured 175.8us.
    no_accum=False,  # TIMING PROBE ONLY: issue every matmul as its own
    # accumulation group (start=stop=True, wrong numerics) to isolate the
    # PSUM accumulate (read-modify-write) cost per column.
    contig_rhs=False,  # TIMING PROBE ONLY: replace the [2,256]-strided rhs
    # with a flat contiguous 512-element slice of x (wrong numerics) to
    # isolate the per-segment AP-restart cost of multi-dim moving operands.
    dedup_ldw=True,  # post-compile: delete InstLdweights whose access
    # pattern equals the immediately preceding (surviving) one on the PE
    # stream. Matmuls are non-self-loading (tile_legalize pairs each with
    # a standalone LDWEIGHTS unconditionally), so the PE array retains
    # weights across consecutive matmuls and duplicate loads are pure
    # overhead (~22ns each measured: FWL-speed load, not hidden behind
    # the stream). With weight_reuse=8 this drops 576 -> 131 LDWEIGHTS
    # (72 unique + 59 kept because they carry a semaphore wait).
):
    """Build + compile the per-core Bass program (identical on all cores).

    mm_dtype: matmul operand dtype. bfloat16 (default) halves DMA/SBUF
    traffic and enables the compiler's fast-weight-load path (FWL is
    disabled for 4-byte operands), hiding LDWEIGHTS behind streaming.
    Accuracy vs the f32 reference is ~2.7e-3 relmax (quantization of both
    operands, fp32 PSUM accumulation), measured offline on the exact
    problem data.
    out_dt: y DMA dtype. bfloat16 halves store traffic (host upcasts);
    adds ~<1e-3 to relmax error.
    split_queues: issue y stores on the ACT HWDGE queue instead of SP, so
    next iteration's x prefetch (SP queue) isn't FIFO-blocked behind this
    iteration's 32 output stores.
    repeat: python-unrolled repetitions of the compute pass (dev timing).
    loop_repeat: hardware For_i repetitions of the whole pass (dev timing).
    """
    nc = bacc.Bacc(
        "TRN2", target_bir_lowering=False, debug=False, num_devices=NCORES
    )
    in_dt = F32 if mm_dtype == mybir.dt.float32r else mm_dtype
    nrep = None
    if loop_repeat == "dynamic":
        # Runtime-controlled repeat count (timing harness): one NEFF serves
        # every rep count. Loaded straight from DRAM into per-engine regs,
        # same mechanism as partition_id.
        nrep = nc.dram_tensor(
            "nrep", [1, 1], mybir.dt.uint32, kind="ExternalInput"
        )
    x = nc.dram_tensor("x", [IN_C, IN_ROWS * W], in_dt, kind="ExternalInput")
    wT = nc.dram_tensor("wT", [9 * IN_C, OUT_C], in_dt, kind="ExternalInput")
    y = nc.dram_tensor(
        "y", [OUT_C, ROWS_PER_CORE * W_OUT], out_dt, kind="ExternalOutput"
    )

    xv = x.rearrange("(b p) (r c) -> p b r c", p=P, c=W)
    wv = wT.rearrange("(b p) m -> p b m", p=P)
    if mm_dtype == mybir.dt.float32r:
        # f32r is bit-compatible with f32; declaring the SBUF tiles f32r
        # (and bitcasting the DMA source) satisfies the walrus requirement
        # that FP32r matmul operands come from an f32r-typed producer.
        xv = xv.bitcast(mm_dtype)
        wv = wv.bitcast(mm_dtype)

    looped = loop_repeat == "dynamic" or loop_repeat > 1
    with tile.TileContext(nc) as tc:
        with ExitStack() as ctx:
            xpool = ctx.enter_context(
                tc.tile_pool(name="xp", bufs=2 if looped else 1)
            )
            wpool = ctx.enter_context(tc.tile_pool(name="wp", bufs=1))
            pspool = ctx.enter_context(
                tc.tile_pool(
                    name="ps",
                    bufs=min(8, 16 // rows_per_mm),
                    space="PSUM",
                )
            )
            opool = ctx.enter_context(
                tc.tile_pool(name="op", bufs=8 if weight_reuse else 4)
            )

            # HAM warmup: the PE clock is gated to 1.2 GHz until ~3.4us of
            # sustained activity. Fill the initial DMA wait (weights + first
            # input chunk) with throwaway fp32 matmuls on a zeroed tile so
            # the real f32r stream starts at the full 2.4 GHz. fp32 avoids
            # the f32r rounded-producer requirement; results are never read.
            warm = wpool.tile([P, P], F32, name="warm")
            nc.gpsimd.memset(warm[:], 0.0)
            wps = pspool.tile([P, rows_per_mm, W_OUT], F32, name="ps", tag="ps")
            for _ in range(12):
                nc.tensor.matmul(
                    wps[:, 0, 0:P],
                    warm[:],
                    warm[:],
                    start=True,
                    stop=True,
                    skip_group_check=True,
                )

            # Split the weight load by out-channel half: the first
            # accumulation group only consumes ocb=0 columns, so compute can
            # start once the first half (~1.2MB) lands instead of waiting for
            # the full 2.3MB transfer; the ocb=1 half streams in behind it.
            w_sb = wpool.tile([P, KB, OUT_C], mm_dtype)
            nc.sync.dma_start(w_sb[:, :, 0:P], wv[:, :, 0:P])
            nc.sync.dma_start(w_sb[:, :, P:OUT_C], wv[:, :, P:OUT_C])

            def _one_pass():
                x_sb = xpool.tile([P, ICB, IN_ROWS, W], mm_dtype, name="x_sb")
                r0 = 0
                while r0 < IN_ROWS:
                    r1 = min(r0 + x_chunk_rows, IN_ROWS)
                    for b in range(ICB):
                        nc.sync.dma_start(
                            x_sb[:, b, r0:r1, :], xv[:, b, r0:r1, :]
                        )
                    r0 = r1
                rmm = rows_per_mm
                ngrp = ROWS_PER_CORE // rmm

                def _emit_out(ps, pr, ocb):
                    if skip_out:
                        return
                    ot = opool.tile([P, rmm * W_OUT], out_dt)
                    nc.vector.tensor_copy(
                        ot[:], ps.rearrange("p a b -> p (a b)")
                    )
                    store_eng = nc.scalar if split_queues else nc.sync
                    store_eng.dma_start(
                        y[
                            ocb * P : (ocb + 1) * P,
                            pr * rmm * W_OUT : (pr + 1) * rmm * W_OUT,
                        ],
                        ot[:],
                    )

                def _mm(ps, pr, ocb, ki):
                    icb, pos = divmod(ki, 9)
                    dy, dx = divmod(pos, 3)
                    kb = 0 if same_weights else pos * ICB + icb
                    lhsT = w_sb[:, kb, ocb * P : (ocb + 1) * P]
                    if contig_rhs:
                        xf = x_sb.rearrange("p b r c -> p (b r c)")
                        n = rmm * W_OUT
                        base = ((pr * KB + ki) * n) % (
                            ICB * IN_ROWS * W - n
                        )
                        rhs = xf[:, base : base + n]
                    else:
                        rhs = x_sb[
                            :,
                            icb,
                            rmm * pr + dy : rmm * pr + dy + rmm,
                            dx : dx + W_OUT,
                        ]
                    nc.tensor.matmul(
                        ps[:, :, :],
                        lhsT,
                        rhs,
                        start=True if no_accum else (ki == 0),
                        stop=True if no_accum else (ki == KB - 1),
                        skip_group_check=no_accum,
                    )

                if weight_reuse:
                    R = weight_reuse
                    assert ngrp % R == 0
                    for ocb in range(OCB):
                        for blk in range(ngrp // R):
                            pss = [
                                pspool.tile(
                                    [P, rmm, W_OUT], F32, name="ps", tag="ps"
                                )
                                for _ in range(R)
                            ]
                            for ki in range(KB):
                                for j in range(R):
                                    _mm(pss[j], blk * R + j, ocb, ki)
                            for j in range(R):
                                _emit_out(pss[j], blk * R + j, ocb)
                elif interleave:
                    for pr in range(ngrp):
                        psa = pspool.tile([P, rmm, W_OUT], F32, name="ps", tag="ps")
                        psb = pspool.tile([P, rmm, W_OUT], F32, name="ps", tag="ps")
                        for ki in range(KB):
                            _mm(psa, pr, 0, ki)
                            _mm(psb, pr, 1, ki)
                        _emit_out(psa, pr, 0)
                        _emit_out(psb, pr, 1)
                else:
                    for pr in range(ngrp):
                        for ocb in range(OCB):
                            ps = pspool.tile([P, rmm, W_OUT], F32, name="ps", tag="ps")
                            for ki in range(KB):
                                _mm(ps, pr, ocb, ki)
                            _emit_out(ps, pr, ocb)

            if loop_repeat == "dynamic":
                nval = nc.values_load(
                    nrep[0:1, 0:1], min_val=1, max_val=10_000_000
                )
                with tc.For_i(0, nval, 1):
                    for _rep in range(repeat):
                        _one_pass()
            elif loop_repeat > 1:
                with tc.For_i(0, loop_repeat, 1):
                    for _rep in range(repeat):
                        _one_pass()
            else:
                for _rep in range(repeat):
                    _one_pass()
    nc.compile()
    if dedup_ldw:
        _dedup_ldweights(nc)
    return nc


def _next_pe_inst(insts, idx):
    """Next PE-engine instruction after index `idx` in the block list
    (other engines' instructions interleave in program order)."""
    for j in range(idx + 1, len(insts)):
        if str(insts[j].engine) == "EngineType.PE":
            return insts[j]
    return None


def _dedup_ldweights(nc):
    """Delete PE InstLdweights that reload the access pattern already in
    the array. tile_legalize pairs EVERY non-f32 matmul with its own
    standalone InstLdweights (no elision), but the matmuls themselves are
    non-self-loading, so after `LDW w; MM; MM; ...` the array still holds
    w and the repeated loads are dead weight. Only sync-free duplicates
    are removed (waits/updates stay where the scheduler put them); any
    self-loading or transpose matmul invalidates the tracked state.
    Engine semaphore counts are unaffected: LDWEIGHTS carries no
    on_update in this program (asserted via has_sync check)."""
    removed = kept = 0
    for f in nc.m.functions:
        for b in f.blocks:
            insts = b.instructions
            out = []
            last_sig = None
            for idx, inst in enumerate(insts):
                tn = type(inst).__name__
                if tn == "InstMatmult":
                    if getattr(inst, "ldweights", False) or getattr(
                        inst, "is_transpose", False
                    ):
                        last_sig = None
                    out.append(inst)
                elif tn == "InstLdweights":
                    si = inst.sync_info
                    n_wait = len(si.on_wait) if si is not None else 0
                    n_upd = len(si.on_update) if si is not None else 0
                    ap = inst.ins[0]
                    c = ap.concise() if callable(ap.concise) else ap.concise
                    sig = (c, getattr(inst, "perf_mode", None))
                    if sig == last_sig and n_upd == 0:
                        if n_wait == 0:
                            removed += 1
                            continue
                        # Single wait (TRN2 limit): migrate it onto the
                        # paired matmul - the next PE instruction - iff
                        # that matmul is wait-free. This inverts
                        # move_matmul_waits_to_ldweights for the dup, so
                        # ordering semantics (wait before the matmul
                        # executes) are preserved.
                        nxt = _next_pe_inst(insts, idx)
                        if (
                            n_wait == 1
                            and nxt is not None
                            and type(nxt).__name__ == "InstMatmult"
                            and (
                                nxt.sync_info is None
                                or len(nxt.sync_info.on_wait) == 0
                            )
                        ):
                            upd = (
                                list(nxt.sync_info.on_update)
                                if nxt.sync_info is not None
                                else []
                            )
                            nxt.sync_info = mybir.SyncInfo(
                                on_wait=list(si.on_wait), on_update=upd
                            )
                            removed += 1
                            continue
                    last_sig = sig
                    kept += 1
                    out.append(inst)
                else:
                    out.append(inst)
            if len(out) != len(insts):
                insts[:] = out
    return removed, kept


_NC_CACHE = {}


def _get_nc():
    if "nc" not in _NC_CACHE:
        _NC_CACHE["nc"] = build()
    return _NC_CACHE["nc"]


def make_in_maps(input, kernelsL, np_dt=ml_dtypes.bfloat16):
    inp = np.asarray(input, dtype=np.float32).reshape(IN_C, H, W).astype(np_dt)
    w = np.asarray(kernelsL, dtype=np.float32)
    # wT[pos*256 + ic, oc] = kernelsL[oc, ic*9 + pos]
    wT = np.ascontiguousarray(
        w.reshape(OUT_C, IN_C, 9)
        .transpose(2, 1, 0)
        .reshape(9 * IN_C, OUT_C)
        .astype(np_dt)
    )
    in_maps = []
    for c in range(NCORES):
        r0 = c * ROWS_PER_CORE
        strip = np.ascontiguousarray(inp[:, r0 : r0 + IN_ROWS, :]).reshape(
            IN_C, IN_ROWS * W
        )
        in_maps.append({"x": strip, "wT": wT})
    return in_maps


def assemble(results):
    out = np.empty((OUT_C, H_OUT, W_OUT), dtype=np.float32)
    for c in range(NCORES):
        out[:, c * ROWS_PER_CORE : (c + 1) * ROWS_PER_CORE, :] = (
            np.asarray(results[c]["y"])
            .astype(np.float32)
            .reshape(OUT_C, ROWS_PER_CORE, W_OUT)
        )
    return out.reshape(1, OUT_C, H_OUT, W_OUT)


def kernel(input, kernelsL):
    in_maps = make_in_maps(input, kernelsL)
    nc = _get_nc()
    res = run_bass_kernel_spmd(nc, in_maps, core_ids=list(range(NCORES)))
    return assemble(res.results)



# revision 19
# speedup vs baseline: 1.1164x; 1.1164x over previous
"""Trainium2 Bass kernel for nn_GCK3x3Layer: 3x3 VALID conv, 256->256 ch, 258x258.

result = kernelsL @ im2col_3x3(input); input (1,256,258,258) f32,
kernelsL (256, 2304) f32 -> output (1, 256, 256, 256) f32.

Strategy: spatial-parallel across 8 NeuronCores. Each core gets a 34-row
input slab (32 output rows + 2 halo rows) and the full weight matrix, and
computes all 256 output channels for its strip via implicit-GEMM:
for each of 9 filter taps and 2 input-channel blocks, a [128,128]x[128,512]
matmul accumulating into PSUM (K = 2304 contraction in 18 chunks of 128,
N = 512 = two output rows of 256 pixels).

Schedule: weight-stationary with LDWEIGHTS dedup. Matmuls are issued
ki-outer / row-pair-inner so 8 consecutive matmuls (into the 8 PSUM
banks) share one weight tile, and a post-compile pass deletes the
duplicate InstLdweights that tile_legalize pairs with every matmul
(576 -> 131). On a cool device this measured 166.5us/pass/core vs
177.3us for the ki-inner order with per-matmul weight loads; on a
heat-soaked device both measure ~205us (thermally-fair interleaved A/B
= parity) because the throttled column rate binds everything.

Measured perf envelope (this axon-tunneled trn2): the tensor engine
sustains ~1.7-1.8 streamed columns/ns cool, ~1.45 heat-soaked (nominal
2.4GHz): probes showed total time tracks total matmul free-dim columns
(294,912/core) regardless of N per matmul (576xN512 == 1152xN256),
rhs AP contiguity, or drain activity - so bf16 direct conv is at the
column-rate floor. fp8 DoubleRow would halve columns but fails the
2e-2 gate (measured e4m3 relmax 0.039; w-split hi+lo 0.026 at 2x
cost; error-compensated 3-term split passes but costs 1.5x columns).
Sustained benching throttles progressively: per-pass estimates drift
+5-10% across 6 timing rounds and ~+15% across a long session.
"""

import os
import sys
from contextlib import ExitStack

import numpy as np

for _p in (
    "/root/.axon_site",
    "/root/.axon_site/_ro/trn_rl_repo",
    "/root/.axon_site/_ro/pypackages",
    "/opt/trn_rl_repo",
):
    if os.path.isdir(_p) and _p not in sys.path:
        sys.path.append(_p)

import ml_dtypes  # noqa: E402

import concourse.bass as bass  # noqa: E402
import concourse.tile as tile  # noqa: E402
from concourse import bacc, mybir  # noqa: E402
from concourse.bass_utils import run_bass_kernel_spmd  # noqa: E402

IN_C = 256
OUT_C = 256
H = 258
W = 258
H_OUT = H - 2  # 256
W_OUT = W - 2  # 256
NCORES = 8
ROWS_PER_CORE = H_OUT // NCORES  # 32
IN_ROWS = ROWS_PER_CORE + 2  # 34
P = 128
ICB = IN_C // P  # 2 input-channel blocks
OCB = OUT_C // P  # 2 output-channel blocks
KB = ICB * 9  # 18 contraction blocks of 128
PAIRS = ROWS_PER_CORE // 2  # 16 output-row pairs (N=512 per matmul)

F32 = mybir.dt.float32


def build(
    mm_dtype=mybir.dt.bfloat16,
    repeat=1,
    x_chunk_rows=6,
    loop_repeat=1,
    out_dt=mybir.dt.bfloat16,
    split_queues=True,
    same_weights=False,  # TIMING PROBE ONLY: reuse one weight tile in all
    # matmuls (wrong numerics) to see if repeated identical LDWEIGHTS get
    # elided / hidden. Never used by kernel().
    rows_per_mm=2,  # output rows per matmul: 2 -> N=512 (one PSUM bank),
    # 4 -> N=1024 (PSUM tile spans two banks, halves matmul count).
    # NOTE: 4 is rejected by the ISA (s3d3_mm_num_elements) - matmul
    # output must fit one PSUM bank. Keep 2.
    skip_out=False,  # TIMING PROBE ONLY: drop PSUM->SBUF copies and
    # output stores (wrong output) to bracket the drain-path cost.
    interleave=False,  # interleave the two ocb accumulation groups of each
    # row-pair (two PSUM banks in flight), halving group boundaries so the
    # PE issue stream has fewer chances to micro-idle (HAM oscillation).
    weight_reuse=8,  # None = original order (ki inner per PSUM group).
    # int R: weight-stationary order - for each contraction block ki, issue
    # R matmuls (R row-pair PSUM banks in flight) sharing one weight tile,
    # so consecutive LDWEIGHTS are identical and dedup_ldw can drop them.
    # R=8 claims all 8 PSUM banks; the bank-reuse wait at each of the 4
    # block boundaries trails the last matmul writing that bank by 7
    # matmuls, so the DVE drains never stall the PE. Measured (with
    # dedup_ldw) 166.5us/pass vs 177.3us for the original order; R=4
    # measured 175.8us.
    no_accum=False,  # TIMING PROBE ONLY: issue every matmul as its own
    # accumulation group (start=stop=True, wrong numerics) to isolate the
    # PSUM accumulate (read-modify-write) cost per column.
    contig_rhs=False,  # TIMING PROBE ONLY: replace the [2,256]-strided rhs
    # with a flat contiguous 512-element slice of x (wrong numerics) to
    # isolate the per-segment AP-restart cost of multi-dim moving operands.
    dedup_ldw=True,  # post-compile: delete InstLdweights whose access
    # pattern equals the immediately preceding (surviving) one on the PE
    # stream. Matmuls are non-self-loading (tile_legalize pairs each with
    # a standalone LDWEIGHTS unconditionally), so the PE array retains
    # weights across consecutive matmuls and duplicate loads are pure
    # overhead (~22ns each measured: FWL-speed load, not hidden behind
    # the stream). With weight_reuse=8 this drops 576 -> 131 LDWEIGHTS
    # (72 unique + 59 kept because they carry a semaphore wait).
):
    """Build + compile the per-core Bass program (identical on all cores).

    mm_dtype: matmul operand dtype. bfloat16 (default) halves DMA/SBUF
    traffic and enables the compiler's fast-weight-load path (FWL is
    disabled for 4-byte operands), hiding LDWEIGHTS behind streaming.
    Accuracy vs the f32 reference is ~2.7e-3 relmax (quantization of both
    operands, fp32 PSUM accumulation), measured offline on the exact
    problem data.
    out_dt: y DMA dtype. bfloat16 halves store traffic (host upcasts);
    adds ~<1e-3 to relmax error.
    split_queues: issue y stores on the ACT HWDGE queue instead of SP, so
    next iteration's x prefetch (SP queue) isn't FIFO-blocked behind this
    iteration's 32 output stores.
    repeat: python-unrolled repetitions of the compute pass (dev timing).
    loop_repeat: hardware For_i repetitions of the whole pass (dev timing).
    """
    nc = bacc.Bacc(
        "TRN2", target_bir_lowering=False, debug=False, num_devices=NCORES
    )
    in_dt = F32 if mm_dtype == mybir.dt.float32r else mm_dtype
    nrep = None
    if loop_repeat == "dynamic":
        # Runtime-controlled repeat count (timing harness): one NEFF serves
        # every rep count. Loaded straight from DRAM into per-engine regs,
        # same mechanism as partition_id.
        nrep = nc.dram_tensor(
            "nrep", [1, 1], mybir.dt.uint32, kind="ExternalInput"
        )
    x = nc.dram_tensor("x", [IN_C, IN_ROWS * W], in_dt, kind="ExternalInput")
    wT = nc.dram_tensor("wT", [9 * IN_C, OUT_C], in_dt, kind="ExternalInput")
    y = nc.dram_tensor(
        "y", [OUT_C, ROWS_PER_CORE * W_OUT], out_dt, kind="ExternalOutput"
    )

    xv = x.rearrange("(b p) (r c) -> p b r c", p=P, c=W)
    wv = wT.rearrange("(b p) m -> p b m", p=P)
    if mm_dtype == mybir.dt.float32r:
        # f32r is bit-compatible with f32; declaring the SBUF tiles f32r
        # (and bitcasting the DMA source) satisfies the walrus requirement
        # that FP32r matmul operands come from an f32r-typed producer.
        xv = xv.bitcast(mm_dtype)
        wv = wv.bitcast(mm_dtype)

    looped = loop_repeat == "dynamic" or loop_repeat > 1
    with tile.TileContext(nc) as tc:
        with ExitStack() as ctx:
            xpool = ctx.enter_context(
                tc.tile_pool(name="xp", bufs=2 if looped else 1)
            )
            wpool = ctx.enter_context(tc.tile_pool(name="wp", bufs=1))
            pspool = ctx.enter_context(
                tc.tile_pool(
                    name="ps",
                    bufs=min(8, 16 // rows_per_mm),
                    space="PSUM",
                )
            )
            opool = ctx.enter_context(
                tc.tile_pool(name="op", bufs=8 if weight_reuse else 4)
            )

            # HAM warmup: the PE clock is gated to 1.2 GHz until ~3.4us of
            # sustained activity. Fill the initial DMA wait (weights + first
            # input chunk) with throwaway fp32 matmuls on a zeroed tile so
            # the real f32r stream starts at the full 2.4 GHz. fp32 avoids
            # the f32r rounded-producer requirement; results are never read.
            warm = wpool.tile([P, P], F32, name="warm")
            nc.gpsimd.memset(warm[:], 0.0)
            wps = pspool.tile([P, rows_per_mm, W_OUT], F32, name="ps", tag="ps")
            for _ in range(12):
                nc.tensor.matmul(
                    wps[:, 0, 0:P],
                    warm[:],
                    warm[:],
                    start=True,
                    stop=True,
                    skip_group_check=True,
                )

            # Split the weight load by out-channel half: the first
            # accumulation group only consumes ocb=0 columns, so compute can
            # start once the first half (~1.2MB) lands instead of waiting for
            # the full 2.3MB transfer; the ocb=1 half streams in behind it.
            w_sb = wpool.tile([P, KB, OUT_C], mm_dtype)
            nc.sync.dma_start(w_sb[:, :, 0:P], wv[:, :, 0:P])
            nc.sync.dma_start(w_sb[:, :, P:OUT_C], wv[:, :, P:OUT_C])

            def _one_pass():
                x_sb = xpool.tile([P, ICB, IN_ROWS, W], mm_dtype, name="x_sb")
                r0 = 0
                while r0 < IN_ROWS:
                    r1 = min(r0 + x_chunk_rows, IN_ROWS)
                    for b in range(ICB):
                        nc.sync.dma_start(
                            x_sb[:, b, r0:r1, :], xv[:, b, r0:r1, :]
                        )
                    r0 = r1
                rmm = rows_per_mm
                ngrp = ROWS_PER_CORE // rmm

                def _emit_out(ps, pr, ocb):
                    if skip_out:
                        return
                    ot = opool.tile([P, rmm * W_OUT], out_dt)
                    nc.vector.tensor_copy(
                        ot[:], ps.rearrange("p a b -> p (a b)")
                    )
                    store_eng = nc.scalar if split_queues else nc.sync
                    store_eng.dma_start(
                        y[
                            ocb * P : (ocb + 1) * P,
                            pr * rmm * W_OUT : (pr + 1) * rmm * W_OUT,
                        ],
                        ot[:],
                    )

                def _mm(ps, pr, ocb, ki):
                    icb, pos = divmod(ki, 9)
                    dy, dx = divmod(pos, 3)
                    kb = 0 if same_weights else pos * ICB + icb
                    lhsT = w_sb[:, kb, ocb * P : (ocb + 1) * P]
                    if contig_rhs:
                        xf = x_sb.rearrange("p b r c -> p (b r c)")
                        n = rmm * W_OUT
                        base = ((pr * KB + ki) * n) % (
                            ICB * IN_ROWS * W - n
                        )
                        rhs = xf[:, base : base + n]
                    else:
                        rhs = x_sb[
                            :,
                            icb,
                            rmm * pr + dy : rmm * pr + dy + rmm,
                            dx : dx + W_OUT,
                        ]
                    nc.tensor.matmul(
                        ps[:, :, :],
                        lhsT,
                        rhs,
                        start=True if no_accum else (ki == 0),
                        stop=True if no_accum else (ki == KB - 1),
                        skip_group_check=no_accum,
                    )

                if weight_reuse:
                    R = weight_reuse
                    assert ngrp % R == 0
                    for ocb in range(OCB):
                        for blk in range(ngrp // R):
                            pss = [
                                pspool.tile(
                                    [P, rmm, W_OUT], F32, name="ps", tag="ps"
                                )
                                for _ in range(R)
                            ]
                            for ki in range(KB):
                                for j in range(R):
                                    _mm(pss[j], blk * R + j, ocb, ki)
                            for j in range(R):
                                _emit_out(pss[j], blk * R + j, ocb)
                elif interleave:
                    for pr in range(ngrp):
                        psa = pspool.tile([P, rmm, W_OUT], F32, name="ps", tag="ps")
                        psb = pspool.tile([P, rmm, W_OUT], F32, name="ps", tag="ps")
                        for ki in range(KB):
                            _mm(psa, pr, 0, ki)
                            _mm(psb, pr, 1, ki)
                        _emit_out(psa, pr, 0)
                        _emit_out(psb, pr, 1)
                else:
                    for pr in range(ngrp):
                        for ocb in range(OCB):
                            ps = pspool.tile([P, rmm, W_OUT], F32, name="ps", tag="ps")
                            for ki in range(KB):
                                _mm(ps, pr, ocb, ki)
                            _emit_out(ps, pr, ocb)

            if loop_repeat == "dynamic":
                nval = nc.values_load(
                    nrep[0:1, 0:1], min_val=1, max_val=10_000_000
                )
                with tc.For_i(0, nval, 1):
                    for _rep in range(repeat):
                        _one_pass()
            elif loop_repeat > 1:
                with tc.For_i(0, loop_repeat, 1):
                    for _rep in range(repeat):
                        _one_pass()
            else:
                for _rep in range(repeat):
                    _one_pass()
    nc.compile()
    if dedup_ldw:
        _dedup_ldweights(nc)
    return nc


def _next_pe_inst(insts, idx):
    """Next PE-engine instruction after index `idx` in the block list
    (other engines' instructions interleave in program order)."""
    for j in range(idx + 1, len(insts)):
        if str(insts[j].engine) == "EngineType.PE":
            return insts[j]
    return None


def _dedup_ldweights(nc):
    """Delete PE InstLdweights that reload the access pattern already in
    the array. tile_legalize pairs EVERY non-f32 matmul with its own
    standalone InstLdweights (no elision), but the matmuls themselves are
    non-self-loading, so after `LDW w; MM; MM; ...` the array still holds
    w and the repeated loads are dead weight. Only sync-free duplicates
    are removed (waits/updates stay where the scheduler put them); any
    self-loading or transpose matmul invalidates the tracked state.
    Engine semaphore counts are unaffected: LDWEIGHTS carries no
    on_update in this program (asserted via has_sync check)."""
    removed = kept = 0
    for f in nc.m.functions:
        for b in f.blocks:
            insts = b.instructions
            out = []
            last_sig = None
            for idx, inst in enumerate(insts):
                tn = type(inst).__name__
                if tn == "InstMatmult":
                    if getattr(inst, "ldweights", False) or getattr(
                        inst, "is_transpose", False
                    ):
                        last_sig = None
                    out.append(inst)
                elif tn == "InstLdweights":
                    si = inst.sync_info
                    n_wait = len(si.on_wait) if si is not None else 0
                    n_upd = len(si.on_update) if si is not None else 0
                    ap = inst.ins[0]
                    c = ap.concise() if callable(ap.concise) else ap.concise
                    sig = (c, getattr(inst, "perf_mode", None))
                    if sig == last_sig and n_upd == 0:
                        if n_wait == 0:
                            removed += 1
                            continue
                        # Single wait (TRN2 limit): migrate it onto the
                        # paired matmul - the next PE instruction - iff
                        # that matmul is wait-free. This inverts
                        # move_matmul_waits_to_ldweights for the dup, so
                        # ordering semantics (wait before the matmul
                        # executes) are preserved.
                        nxt = _next_pe_inst(insts, idx)
                        if (
                            n_wait == 1
                            and nxt is not None
                            and type(nxt).__name__ == "InstMatmult"
                            and (
                                nxt.sync_info is None
                                or len(nxt.sync_info.on_wait) == 0
                            )
                        ):
                            upd = (
                                list(nxt.sync_info.on_update)
                                if nxt.sync_info is not None
                                else []
                            )
                            nxt.sync_info = mybir.SyncInfo(
                                on_wait=list(si.on_wait), on_update=upd
                            )
                            removed += 1
                            continue
                    last_sig = sig
                    kept += 1
                    out.append(inst)
                else:
                    out.append(inst)
            if len(out) != len(insts):
                insts[:] = out
    return removed, kept


_NC_CACHE = {}


def _get_nc():
    if "nc" not in _NC_CACHE:
        _NC_CACHE["nc"] = build()
    return _NC_CACHE["nc"]


def make_in_maps(input, kernelsL, np_dt=ml_dtypes.bfloat16):
    inp = np.asarray(input, dtype=np.float32).reshape(IN_C, H, W).astype(np_dt)
    w = np.asarray(kernelsL, dtype=np.float32)
    # wT[pos*256 + ic, oc] = kernelsL[oc, ic*9 + pos]
    wT = np.ascontiguousarray(
        w.reshape(OUT_C, IN_C, 9)
        .transpose(2, 1, 0)
        .reshape(9 * IN_C, OUT_C)
        .astype(np_dt)
    )
    in_maps = []
    for c in range(NCORES):
        r0 = c * ROWS_PER_CORE
        strip = np.ascontiguousarray(inp[:, r0 : r0 + IN_ROWS, :]).reshape(
            IN_C, IN_ROWS * W
        )
        in_maps.append({"x": strip, "wT": wT})
    return in_maps


def assemble(results):
    out = np.empty((OUT_C, H_OUT, W_OUT), dtype=np.float32)
    for c in range(NCORES):
        out[:, c * ROWS_PER_CORE : (c + 1) * ROWS_PER_CORE, :] = (
            np.asarray(results[c]["y"])
            .astype(np.float32)
            .reshape(OUT_C, ROWS_PER_CORE, W_OUT)
        )
    return out.reshape(1, OUT_C, H_OUT, W_OUT)


def kernel(input, kernelsL):
    in_maps = make_in_maps(input, kernelsL)
    nc = _get_nc()
    res = run_bass_kernel_spmd(nc, in_maps, core_ids=list(range(NCORES)))
    return assemble(res.results)



# revision 22
# speedup vs baseline: 1.1457x; 1.0262x over previous
"""Trainium2 Bass kernel for nn_GCK3x3Layer: 3x3 VALID conv, 256->256 ch, 258x258.

result = kernelsL @ im2col_3x3(input); input (1,256,258,258) f32,
kernelsL (256, 2304) f32 -> output (1, 256, 256, 256) f32.

Strategy: spatial-parallel across 8 NeuronCores. Each core gets a 34-row
input slab (32 output rows + 2 halo rows) and the full weight matrix, and
computes all 256 output channels for its strip via implicit-GEMM:
for each of 9 filter taps and 2 input-channel blocks, a [128,128]x[128,512]
matmul accumulating into PSUM (K = 2304 contraction in 18 chunks of 128,
N = 512 = two output rows of 256 pixels).

Measured perf envelope (this axon-tunneled trn2): the tensor engine
sustains ~1.7-1.8 streamed columns/ns cool, ~1.45 heat-soaked (nominal
2.4GHz): probes showed total time tracks total matmul free-dim columns
(294,912/core) regardless of N per matmul (576xN512 == 1152xN256),
rhs AP contiguity, or drain activity - so this bf16 direct conv sits
AT the column-rate floor and the schedule below is the measured
optimum. Alternatives ruled out by thermally-fair interleaved A/B
(probe_ab.py) or analysis:
- weight-stationary order + LDWEIGHTS dedup (weight_reuse=8,
  dedup_ldw=True): 1.021-1.026x SLOWER (loads already hidden by the
  PE reorder window; per-matmul PSUM-bank rotation costs ~2%).
- fp8 DoubleRow would halve columns but fails the 2e-2 gate (measured
  e4m3 relmax 0.039; w-split hi+lo 0.026 at 2x cost; 3-term split
  passes but costs 1.5x columns). int8 not exposed by Bass. Winograd
  transforms are vector-bound on this engine mix.
- rows_per_mm=1 (N=256), contiguous-rhs, R=4, 2-pass unroll: all
  neutral-to-worse (see build() probe flags).
Sustained benching throttles progressively: per-pass estimates drift
+5-10% across 6 timing rounds and ~+15% across a heat-soaked session
(thermal time constant is minutes), so cross-run comparisons need
interleaved A/B.
"""

import os
import sys
from contextlib import ExitStack

import numpy as np

for _p in (
    "/root/.axon_site",
    "/root/.axon_site/_ro/trn_rl_repo",
    "/root/.axon_site/_ro/pypackages",
    "/opt/trn_rl_repo",
):
    if os.path.isdir(_p) and _p not in sys.path:
        sys.path.append(_p)

import ml_dtypes  # noqa: E402

import concourse.bass as bass  # noqa: E402
import concourse.tile as tile  # noqa: E402
from concourse import bacc, mybir  # noqa: E402
from concourse.bass_utils import run_bass_kernel_spmd  # noqa: E402

IN_C = 256
OUT_C = 256
H = 258
W = 258
H_OUT = H - 2  # 256
W_OUT = W - 2  # 256
NCORES = 8
ROWS_PER_CORE = H_OUT // NCORES  # 32
IN_ROWS = ROWS_PER_CORE + 2  # 34
P = 128
ICB = IN_C // P  # 2 input-channel blocks
OCB = OUT_C // P  # 2 output-channel blocks
KB = ICB * 9  # 18 contraction blocks of 128
PAIRS = ROWS_PER_CORE // 2  # 16 output-row pairs (N=512 per matmul)

F32 = mybir.dt.float32


def build(
    mm_dtype=mybir.dt.bfloat16,
    repeat=1,
    x_chunk_rows=6,
    loop_repeat=1,
    out_dt=mybir.dt.bfloat16,
    split_queues=True,
    same_weights=False,  # TIMING PROBE ONLY: reuse one weight tile in all
    # matmuls (wrong numerics) to see if repeated identical LDWEIGHTS get
    # elided / hidden. Never used by kernel().
    rows_per_mm=2,  # output rows per matmul: 2 -> N=512 (one PSUM bank),
    # 4 -> N=1024 (PSUM tile spans two banks, halves matmul count).
    # NOTE: 4 is rejected by the ISA (s3d3_mm_num_elements) - matmul
    # output must fit one PSUM bank. Keep 2.
    skip_out=False,  # TIMING PROBE ONLY: drop PSUM->SBUF copies and
    # output stores (wrong output) to bracket the drain-path cost.
    interleave=False,  # interleave the two ocb accumulation groups of each
    # row-pair (two PSUM banks in flight), halving group boundaries so the
    # PE issue stream has fewer chances to micro-idle (HAM oscillation).
    weight_reuse=None,  # None = default order (ki inner per PSUM group).
    # int R: weight-stationary order - for each contraction block ki, issue
    # R matmuls (R row-pair PSUM banks in flight) sharing one weight tile,
    # so consecutive LDWEIGHTS are identical and dedup_ldw can drop them.
    # VERDICT: thermally-fair interleaved A/B (probe_ab.py) measured
    # R=8+dedup at 1.021-1.026x the default order (2 sessions, hot and
    # cool) - the per-matmul LDWEIGHTS are evidently already hidden by
    # the PE's 64-deep reorder window, and rotating PSUM banks every
    # matmul costs ~2%. Early single-run "wins" were thermal luck.
    # Keep None.
    no_accum=False,  # TIMING PROBE ONLY: issue every matmul as its own
    # accumulation group (start=stop=True, wrong numerics) to isolate the
    # PSUM accumulate (read-modify-write) cost per column.
    contig_rhs=False,  # TIMING PROBE ONLY: replace the [2,256]-strided rhs
    # with a flat contiguous 512-element slice of x (wrong numerics) to
    # isolate the per-segment AP-restart cost of multi-dim moving operands.
    dedup_ldw=False,  # post-compile: delete InstLdweights whose access
    # pattern equals the immediately preceding (surviving) one on the PE
    # stream. Matmuls are non-self-loading (tile_legalize pairs each with
    # a standalone LDWEIGHTS unconditionally), so the PE array retains
    # weights across consecutive matmuls and duplicate loads are
    # redundant; with weight_reuse=8 this drops 576 -> 131 LDWEIGHTS
    # (72 unique + 59 kept because they carry a semaphore wait), and HW
    # output is bit-identical (verified, rel err 0.0035459 unchanged).
    # But fair A/B shows no win (loads were already hidden) - see
    # weight_reuse above. Off by default.
):
    """Build + compile the per-core Bass program (identical on all cores).

    mm_dtype: matmul operand dtype. bfloat16 (default) halves DMA/SBUF
    traffic and enables the compiler's fast-weight-load path (FWL is
    disabled for 4-byte operands), hiding LDWEIGHTS behind streaming.
    Accuracy vs the f32 reference is ~2.7e-3 relmax (quantization of both
    operands, fp32 PSUM accumulation), measured offline on the exact
    problem data.
    out_dt: y DMA dtype. bfloat16 halves store traffic (host upcasts);
    adds ~<1e-3 to relmax error.
    split_queues: issue y stores on the ACT HWDGE queue instead of SP, so
    next iteration's x prefetch (SP queue) isn't FIFO-blocked behind this
    iteration's 32 output stores.
    repeat: python-unrolled repetitions of the compute pass (dev timing).
    loop_repeat: hardware For_i repetitions of the whole pass (dev timing).
    """
    nc = bacc.Bacc(
        "TRN2", target_bir_lowering=False, debug=False, num_devices=NCORES
    )
    in_dt = F32 if mm_dtype == mybir.dt.float32r else mm_dtype
    nrep = None
    if loop_repeat == "dynamic":
        # Runtime-controlled repeat count (timing harness): one NEFF serves
        # every rep count. Loaded straight from DRAM into per-engine regs,
        # same mechanism as partition_id.
        nrep = nc.dram_tensor(
            "nrep", [1, 1], mybir.dt.uint32, kind="ExternalInput"
        )
    x = nc.dram_tensor("x", [IN_C, IN_ROWS * W], in_dt, kind="ExternalInput")
    wT = nc.dram_tensor("wT", [9 * IN_C, OUT_C], in_dt, kind="ExternalInput")
    y = nc.dram_tensor(
        "y", [OUT_C, ROWS_PER_CORE * W_OUT], out_dt, kind="ExternalOutput"
    )

    xv = x.rearrange("(b p) (r c) -> p b r c", p=P, c=W)
    wv = wT.rearrange("(b p) m -> p b m", p=P)
    if mm_dtype == mybir.dt.float32r:
        # f32r is bit-compatible with f32; declaring the SBUF tiles f32r
        # (and bitcasting the DMA source) satisfies the walrus requirement
        # that FP32r matmul operands come from an f32r-typed producer.
        xv = xv.bitcast(mm_dtype)
        wv = wv.bitcast(mm_dtype)

    looped = loop_repeat == "dynamic" or loop_repeat > 1
    with tile.TileContext(nc) as tc:
        with ExitStack() as ctx:
            xpool = ctx.enter_context(
                tc.tile_pool(name="xp", bufs=2 if looped else 1)
            )
            wpool = ctx.enter_context(tc.tile_pool(name="wp", bufs=1))
            pspool = ctx.enter_context(
                tc.tile_pool(
                    name="ps",
                    bufs=min(8, 16 // rows_per_mm),
                    space="PSUM",
                )
            )
            opool = ctx.enter_context(
                tc.tile_pool(name="op", bufs=8 if weight_reuse else 4)
            )

            # HAM warmup: the PE clock is gated to 1.2 GHz until ~3.4us of
            # sustained activity. Fill the initial DMA wait (weights + first
            # input chunk) with throwaway fp32 matmuls on a zeroed tile so
            # the real f32r stream starts at the full 2.4 GHz. fp32 avoids
            # the f32r rounded-producer requirement; results are never read.
            warm = wpool.tile([P, P], F32, name="warm")
            nc.gpsimd.memset(warm[:], 0.0)
            wps = pspool.tile([P, rows_per_mm, W_OUT], F32, name="ps", tag="ps")
            for _ in range(12):
                nc.tensor.matmul(
                    wps[:, 0, 0:P],
                    warm[:],
                    warm[:],
                    start=True,
                    stop=True,
                    skip_group_check=True,
                )

            # Split the weight load by out-channel half: the first
            # accumulation group only consumes ocb=0 columns, so compute can
            # start once the first half (~1.2MB) lands instead of waiting for
            # the full 2.3MB transfer; the ocb=1 half streams in behind it.
            w_sb = wpool.tile([P, KB, OUT_C], mm_dtype)
            nc.sync.dma_start(w_sb[:, :, 0:P], wv[:, :, 0:P])
            nc.sync.dma_start(w_sb[:, :, P:OUT_C], wv[:, :, P:OUT_C])

            def _one_pass():
                x_sb = xpool.tile([P, ICB, IN_ROWS, W], mm_dtype, name="x_sb")
                r0 = 0
                while r0 < IN_ROWS:
                    r1 = min(r0 + x_chunk_rows, IN_ROWS)
                    for b in range(ICB):
                        nc.sync.dma_start(
                            x_sb[:, b, r0:r1, :], xv[:, b, r0:r1, :]
                        )
                    r0 = r1
                rmm = rows_per_mm
                ngrp = ROWS_PER_CORE // rmm

                def _emit_out(ps, pr, ocb):
                    if skip_out:
                        return
                    ot = opool.tile([P, rmm * W_OUT], out_dt)
                    nc.vector.tensor_copy(
                        ot[:], ps.rearrange("p a b -> p (a b)")
                    )
                    store_eng = nc.scalar if split_queues else nc.sync
                    store_eng.dma_start(
                        y[
                            ocb * P : (ocb + 1) * P,
                            pr * rmm * W_OUT : (pr + 1) * rmm * W_OUT,
                        ],
                        ot[:],
                    )

                def _mm(ps, pr, ocb, ki):
                    icb, pos = divmod(ki, 9)
                    dy, dx = divmod(pos, 3)
                    kb = 0 if same_weights else pos * ICB + icb
                    lhsT = w_sb[:, kb, ocb * P : (ocb + 1) * P]
                    if contig_rhs:
                        xf = x_sb.rearrange("p b r c -> p (b r c)")
                        n = rmm * W_OUT
                        base = ((pr * KB + ki) * n) % (
                            ICB * IN_ROWS * W - n
                        )
                        rhs = xf[:, base : base + n]
                    else:
                        rhs = x_sb[
                            :,
                            icb,
                            rmm * pr + dy : rmm * pr + dy + rmm,
                            dx : dx + W_OUT,
                        ]
                    nc.tensor.matmul(
                        ps[:, :, :],
                        lhsT,
                        rhs,
                        start=True if no_accum else (ki == 0),
                        stop=True if no_accum else (ki == KB - 1),
                        skip_group_check=no_accum,
                    )

                if weight_reuse:
                    R = weight_reuse
                    assert ngrp % R == 0
                    for ocb in range(OCB):
                        for blk in range(ngrp // R):
                            pss = [
                                pspool.tile(
                                    [P, rmm, W_OUT], F32, name="ps", tag="ps"
                                )
                                for _ in range(R)
                            ]
                            for ki in range(KB):
                                for j in range(R):
                                    _mm(pss[j], blk * R + j, ocb, ki)
                            for j in range(R):
                                _emit_out(pss[j], blk * R + j, ocb)
                elif interleave:
                    for pr in range(ngrp):
                        psa = pspool.tile([P, rmm, W_OUT], F32, name="ps", tag="ps")
                        psb = pspool.tile([P, rmm, W_OUT], F32, name="ps", tag="ps")
                        for ki in range(KB):
                            _mm(psa, pr, 0, ki)
                            _mm(psb, pr, 1, ki)
                        _emit_out(psa, pr, 0)
                        _emit_out(psb, pr, 1)
                else:
                    for pr in range(ngrp):
                        for ocb in range(OCB):
                            ps = pspool.tile([P, rmm, W_OUT], F32, name="ps", tag="ps")
                            for ki in range(KB):
                                _mm(ps, pr, ocb, ki)
                            _emit_out(ps, pr, ocb)

            if loop_repeat == "dynamic":
                nval = nc.values_load(
                    nrep[0:1, 0:1], min_val=1, max_val=10_000_000
                )
                with tc.For_i(0, nval, 1):
                    for _rep in range(repeat):
                        _one_pass()
            elif loop_repeat > 1:
                with tc.For_i(0, loop_repeat, 1):
                    for _rep in range(repeat):
                        _one_pass()
            else:
                for _rep in range(repeat):
                    _one_pass()
    nc.compile()
    if dedup_ldw:
        _dedup_ldweights(nc)
    return nc


def _next_pe_inst(insts, idx):
    """Next PE-engine instruction after index `idx` in the block list
    (other engines' instructions interleave in program order)."""
    for j in range(idx + 1, len(insts)):
        if str(insts[j].engine) == "EngineType.PE":
            return insts[j]
    return None


def _dedup_ldweights(nc):
    """Delete PE InstLdweights that reload the access pattern already in
    the array. tile_legalize pairs EVERY non-f32 matmul with its own
    standalone InstLdweights (no elision), but the matmuls themselves are
    non-self-loading, so after `LDW w; MM; MM; ...` the array still holds
    w and the repeated loads are dead weight. Only sync-free duplicates
    are removed (waits/updates stay where the scheduler put them); any
    self-loading or transpose matmul invalidates the tracked state.
    Engine semaphore counts are unaffected: LDWEIGHTS carries no
    on_update in this program (asserted via has_sync check)."""
    removed = kept = 0
    for f in nc.m.functions:
        for b in f.blocks:
            insts = b.instructions
            out = []
            last_sig = None
            for idx, inst in enumerate(insts):
                tn = type(inst).__name__
                if tn == "InstMatmult":
                    if getattr(inst, "ldweights", False) or getattr(
                        inst, "is_transpose", False
                    ):
                        last_sig = None
                    out.append(inst)
                elif tn == "InstLdweights":
                    si = inst.sync_info
                    n_wait = len(si.on_wait) if si is not None else 0
                    n_upd = len(si.on_update) if si is not None else 0
                    ap = inst.ins[0]
                    c = ap.concise() if callable(ap.concise) else ap.concise
                    sig = (c, getattr(inst, "perf_mode", None))
                    if sig == last_sig and n_upd == 0:
                        if n_wait == 0:
                            removed += 1
                            continue
                        # Single wait (TRN2 limit): migrate it onto the
                        # paired matmul - the next PE instruction - iff
                        # that matmul is wait-free. This inverts
                        # move_matmul_waits_to_ldweights for the dup, so
                        # ordering semantics (wait before the matmul
                        # executes) are preserved.
                        nxt = _next_pe_inst(insts, idx)
                        if (
                            n_wait == 1
                            and nxt is not None
                            and type(nxt).__name__ == "InstMatmult"
                            and (
                                nxt.sync_info is None
                                or len(nxt.sync_info.on_wait) == 0
                            )
                        ):
                            upd = (
                                list(nxt.sync_info.on_update)
                                if nxt.sync_info is not None
                                else []
                            )
                            nxt.sync_info = mybir.SyncInfo(
                                on_wait=list(si.on_wait), on_update=upd
                            )
                            removed += 1
                            continue
                    last_sig = sig
                    kept += 1
                    out.append(inst)
                else:
                    out.append(inst)
            if len(out) != len(insts):
                insts[:] = out
    return removed, kept


_NC_CACHE = {}


def _get_nc():
    if "nc" not in _NC_CACHE:
        _NC_CACHE["nc"] = build()
    return _NC_CACHE["nc"]


def make_in_maps(input, kernelsL, np_dt=ml_dtypes.bfloat16):
    inp = np.asarray(input, dtype=np.float32).reshape(IN_C, H, W).astype(np_dt)
    w = np.asarray(kernelsL, dtype=np.float32)
    # wT[pos*256 + ic, oc] = kernelsL[oc, ic*9 + pos]
    wT = np.ascontiguousarray(
        w.reshape(OUT_C, IN_C, 9)
        .transpose(2, 1, 0)
        .reshape(9 * IN_C, OUT_C)
        .astype(np_dt)
    )
    in_maps = []
    for c in range(NCORES):
        r0 = c * ROWS_PER_CORE
        strip = np.ascontiguousarray(inp[:, r0 : r0 + IN_ROWS, :]).reshape(
            IN_C, IN_ROWS * W
        )
        in_maps.append({"x": strip, "wT": wT})
    return in_maps


def assemble(results):
    out = np.empty((OUT_C, H_OUT, W_OUT), dtype=np.float32)
    for c in range(NCORES):
        out[:, c * ROWS_PER_CORE : (c + 1) * ROWS_PER_CORE, :] = (
            np.asarray(results[c]["y"])
            .astype(np.float32)
            .reshape(OUT_C, ROWS_PER_CORE, W_OUT)
        )
    return out.reshape(1, OUT_C, H_OUT, W_OUT)


def kernel(input, kernelsL):
    in_maps = make_in_maps(input, kernelsL)
    nc = _get_nc()
    res = run_bass_kernel_spmd(nc, in_maps, core_ids=list(range(NCORES)))
    return assemble(res.results)

